# revision 21
# baseline (speedup 1.0000x reference)
import sys

sys.path.insert(0, "/opt/trn_rl_repo")
import numpy as np

try:
    import antenv.axon_hooks  # noqa: F401
except ImportError:
    # bass_utils hard-imports this when BASS_TRACE is set; stub it so the
    # run degrades to no-trace instead of crashing.
    import types

    _m = types.ModuleType("antenv.axon_hooks")
    _m._hook = None
    _m.get_axon_ntff_profile_hook = lambda: _m._hook

    def _set_hook(h):
        _m._hook = h

    _m.set_axon_ntff_profile_hook = _set_hook
    import antenv

    sys.modules["antenv.axon_hooks"] = _m
    antenv.axon_hooks = _m

B = 65536
FD = 267
H = 256
L = 64
KC = 1024
NCORES = 8
RPC = B // NCORES          # rows per core: 8192
CH = 512                   # rows per chunk
NCHUNK = RPC // CH         # 16
NSUB = CH // 128           # 4

# encoder input s=[x;c] (534 rows) partition-tile layout; all tiles
# partition-0 aligned (engines need 32-aligned partition starts)
S_SIZES = [128, 128, 11, 128, 128, 11]
# t0=x[0:128] t1=x[128:256] t2=x[256:267] t3=c[0:128] t4=c[128:256] t5=c[256:267]
# decoder K-chunks: q_st(64), c[0:128), c[128:256), c[256:267)
DEC_K = [64, 128, 128, 11]
OUT_M = [128, 128, 11]


def _blob_layout():
    lay = {}
    off = 0

    def add(nm, p, w):
        nonlocal off
        lay[nm] = (p, off, w)
        off += w

    # ---- fp32 region ----
    for i in range(6):
        for m in range(2):
            add(f"fc1T_{i}_{m}", S_SIZES[i], 128)
    for i in range(2):
        for m in range(2):
            add(f"fc2T_{i}_{m}", 128, 128)
    for i in range(2):
        for m in range(2):
            add(f"fc3T_{i}_{m}", 128, 128)
    for i in range(2):
        add(f"muT_{i}", 128, 64)
    add("embed_ext", 65, 1024)
    add("I128", 128, 128)
    add("iota", 128, 8)
    for m in range(2):
        add(f"fc1b_{m}", 128, 1)
        add(f"fc2b_{m}", 128, 1)
        add(f"fc3b_{m}", 128, 1)
        add(f"fc5b_{m}", 128, 1)
        add(f"fc4b_{m}", 128, 1)
        add(f"fc6b_{m}", 128, 1)
    add("mub", 64, 1)
    for m in range(3):
        add(f"outb_{m}", OUT_M[m], 1)
    r0 = off
    # ---- f32r region (rounded on-chip via one Act copy) ----
    for i in range(8):
        add(f"embedT_{i}", 128, 64)
    for ki in range(4):
        for m in range(2):
            add(f"fc4T_{ki}_{m}", DEC_K[ki], 128)
    for i in range(2):
        for m in range(2):
            add(f"fc5T_{i}_{m}", 128, 128)
    for i in range(2):
        for m in range(2):
            add(f"fc6T_{i}_{m}", 128, 128)
    for i in range(2):
        for m in range(3):
            add(f"outT_{i}_{m}", 128, OUT_M[m])
    add("ones", 128, 128)
    return lay, r0, off


_CACHE = {}


def _build_nc():
    import concourse.bacc as bacc
    import concourse.tile as tile
    import concourse.mybir as mybir

    dt = mybir.dt
    AF = mybir.ActivationFunctionType
    ALU = mybir.AluOpType
    AX = mybir.AxisListType

    lay, R0, NW = _blob_layout()
    nc = bacc.Bacc()
    xT = nc.declare_dram_parameter("xT", [FD, RPC], dt.float32, isOutput=False)
    cT = nc.declare_dram_parameter("cT", [FD, RPC], dt.float32, isOutput=False)
    wb = nc.declare_dram_parameter("wb", [128, NW], dt.float32, isOutput=False)
    reconT = nc.declare_dram_parameter("reconT", [FD, RPC], dt.float32, isOutput=True)
    musq = nc.declare_dram_parameter("musq", [L, NCHUNK], dt.float32, isOutput=True)
    maxv = nc.declare_dram_parameter(
        "maxv", [128, NCHUNK * NSUB], dt.float32, isOutput=True
    )
    counts = nc.declare_dram_parameter(
        "counts", [128, NCHUNK * 8], dt.float32, isOutput=True
    )

    with tile.TileContext(nc) as tc:
        with tc.tile_pool(name="const", bufs=1) as cpool, \
             tc.tile_pool(name="act", bufs=2) as apool, \
             tc.tile_pool(name="small", bufs=2) as spool, \
             tc.tile_pool(name="stat", bufs=1) as stpool, \
             tc.tile_pool(name="ps", bufs=2, space="PSUM") as ps, \
             tc.tile_pool(name="psq", bufs=1, space="PSUM") as psq, \
             tc.tile_pool(name="pss", bufs=2, space="PSUM") as pss:

            wb_sb = cpool.tile([128, NW], dt.float32)
            nc.sync.dma_start(out=wb_sb[:], in_=wb[:, :])
            WRW = NW - R0
            wr_sb = cpool.tile([128, WRW], dt.float32r)
            nc.scalar.copy(out=wr_sb[:], in_=wb_sb[:, R0:NW])

            def W32(nm):
                p, o, w = lay[nm]
                return wb_sb[0:p, o:o + w]

            def WR(nm):
                p, o, w = lay[nm]
                return wr_sb[0:p, o - R0:o - R0 + w]

            musq_buf = stpool.tile([L, NCHUNK], dt.float32)
            maxv_buf = stpool.tile([128, NCHUNK * NSUB], dt.float32)
            counts_buf = stpool.tile([128, NCHUNK * 8], dt.float32)

            for j in range(NCHUNK):
                col = slice(j * CH, (j + 1) * CH)
                t = [apool.tile([S_SIZES[i], CH], dt.float32, name=f"s_{i}")
                     for i in range(6)]
                nc.sync.dma_start(out=t[0][:], in_=xT[0:128, col])
                nc.sync.dma_start(out=t[1][:], in_=xT[128:256, col])
                nc.sync.dma_start(out=t[2][:], in_=xT[256:267, col])
                nc.sync.dma_start(out=t[3][:], in_=cT[0:128, col])
                nc.sync.dma_start(out=t[4][:], in_=cT[128:256, col])
                nc.sync.dma_start(out=t[5][:], in_=cT[256:267, col])

                # encoder L1: [534]->256, relu
                h1 = []
                for m in range(2):
                    p = ps.tile([128, CH], dt.float32)
                    for i in range(6):
                        nc.tensor.matmul(
                            p[:], W32(f"fc1T_{i}_{m}"), t[i][:],
                            start=(i == 0), stop=(i == 5),
                        )
                    h = apool.tile([128, CH], dt.float32, name=f"h1_{m}")
                    nc.scalar.activation(h[:], p[:], AF.Relu, bias=W32(f"fc1b_{m}"))
                    h1.append(h)
                # L2
                h2 = []
                for m in range(2):
                    p = ps.tile([128, CH], dt.float32)
                    for i in range(2):
                        nc.tensor.matmul(
                            p[:], W32(f"fc2T_{i}_{m}"), h1[i][:],
                            start=(i == 0), stop=(i == 1),
                        )
                    h = apool.tile([128, CH], dt.float32, name=f"h2_{m}")
                    nc.scalar.activation(h[:], p[:], AF.Relu, bias=W32(f"fc2b_{m}"))
                    h2.append(h)
                # L3
                h3 = []
                for m in range(2):
                    p = ps.tile([128, CH], dt.float32)
                    for i in range(2):
                        nc.tensor.matmul(
                            p[:], W32(f"fc3T_{i}_{m}"), h2[i][:],
                            start=(i == 0), stop=(i == 1),
                        )
                    h = apool.tile([128, CH], dt.float32, name=f"h3_{m}")
                    nc.scalar.activation(h[:], p[:], AF.Relu, bias=W32(f"fc3b_{m}"))
                    h3.append(h)
                # mu: 256->64 (no relu)
                pmb = psq.tile([128, CH], dt.float32, name="pmb")
                pmu = pmb[0:64, :]
                for i in range(2):
                    nc.tensor.matmul(
                        pmu, W32(f"muT_{i}"), h3[i][:],
                        start=(i == 0), stop=(i == 1),
                    )
                muext = apool.tile([65, CH], dt.float32)
                nc.scalar.activation(muext[0:64, :], pmu, AF.Identity,
                                     bias=W32("mub"))
                nc.vector.memset(muext[64:65, :], 1.0)
                sqscr = apool.tile([64, CH], dt.float32)
                nc.scalar.activation(
                    sqscr[:], pmu, AF.Square, bias=W32("mub"),
                    accum_out=musq_buf[:, j:j + 1],
                )

                # scores + argmax per 128-row subtile
                diag = apool.tile([128, CH], dt.float32r)
                emb = W32("embed_ext")
                for s in range(NSUB):
                    psc = pss.tile([128, 1024], dt.float32)
                    lhs = muext[:, s * 128:(s + 1) * 128]
                    nc.tensor.matmul(psc[:, 0:512], lhs, emb[:, 0:512],
                                     start=True, stop=True)
                    nc.tensor.matmul(psc[:, 512:1024], lhs, emb[:, 512:1024],
                                     start=True, stop=True)
                    mx8 = spool.tile([128, 8], dt.float32)
                    nc.vector.max(out=mx8[:], in_=psc[:])
                    ix8 = spool.tile([128, 8], dt.uint32)
                    nc.vector.max_index(out=ix8[:], in_max=mx8[:], in_values=psc[:])
                    cix = j * NSUB + s
                    nc.vector.tensor_copy(out=maxv_buf[:, cix:cix + 1],
                                          in_=mx8[:, 0:1])
                    ixf = spool.tile([128, 1], dt.float32)
                    nc.vector.tensor_copy(out=ixf[:], in_=ix8[:, 0:1])
                    nc.vector.tensor_scalar(
                        out=diag[:, s * 128:(s + 1) * 128], in0=W32("I128"),
                        scalar1=ixf[:], scalar2=None, op0=ALU.mult,
                    )
                # broadcast idx to [128, CH] (reuses pmu's slot; pmu is dead)
                pbc = psq.tile([128, CH], dt.float32, name="pmb")
                nc.tensor.matmul(pbc[:], WR("ones"), diag[:], start=True, stop=True)

                # onehot chunks, counts, quantize
                pq = psq.tile([64, CH], dt.float32, name="pq")
                iota = W32("iota")
                for cc in range(8):
                    oh = apool.tile([128, CH], dt.float32r)
                    nc.vector.tensor_scalar(
                        out=oh[:], in0=pbc[:], scalar1=iota[:, cc:cc + 1],
                        scalar2=None, op0=ALU.is_equal,
                    )
                    ci = j * 8 + cc
                    nc.vector.tensor_reduce(
                        out=counts_buf[:, ci:ci + 1], in_=oh[:], axis=AX.X,
                        op=ALU.add,
                    )
                    nc.tensor.matmul(
                        pq[:], WR(f"embedT_{cc}"), oh[:],
                        start=(cc == 0), stop=(cc == 7),
                    )
                qst = apool.tile([64, CH], dt.float32r)
                nc.scalar.copy(out=qst[:], in_=pq[:])
                cr0 = apool.tile([128, CH], dt.float32r)
                nc.scalar.copy(out=cr0[:], in_=t[3][:])
                cr1 = apool.tile([128, CH], dt.float32r)
                nc.scalar.copy(out=cr1[:], in_=t[4][:])
                cr2 = apool.tile([11, CH], dt.float32r)
                nc.scalar.copy(out=cr2[:], in_=t[5][:])
                dec_rhs = [qst, cr0, cr1, cr2]

                # decoder L1
                h4 = []
                for m in range(2):
                    p = ps.tile([128, CH], dt.float32)
                    for ki in range(4):
                        nc.tensor.matmul(
                            p[:], WR(f"fc4T_{ki}_{m}"), dec_rhs[ki][:],
                            start=(ki == 0), stop=(ki == 3),
                        )
                    h = apool.tile([128, CH], dt.float32r, name=f"h4_{m}")
                    nc.scalar.activation(h[:], p[:], AF.Relu, bias=W32(f"fc4b_{m}"))
                    h4.append(h)
                # L2
                h5 = []
                for m in range(2):
                    p = ps.tile([128, CH], dt.float32)
                    for i in range(2):
                        nc.tensor.matmul(
                            p[:], WR(f"fc5T_{i}_{m}"), h4[i][:],
                            start=(i == 0), stop=(i == 1),
                        )
                    h = apool.tile([128, CH], dt.float32r, name=f"h5_{m}")
                    nc.scalar.activation(h[:], p[:], AF.Relu, bias=W32(f"fc5b_{m}"))
                    h5.append(h)
                # L3: fc6
                h6 = []
                for m in range(2):
                    p = ps.tile([128, CH], dt.float32)
                    for i in range(2):
                        nc.tensor.matmul(
                            p[:], WR(f"fc6T_{i}_{m}"), h5[i][:],
                            start=(i == 0), stop=(i == 1),
                        )
                    h = apool.tile([128, CH], dt.float32r, name=f"h6_{m}")
                    nc.scalar.activation(h[:], p[:], AF.Relu, bias=W32(f"fc6b_{m}"))
                    h6.append(h)
                # L4 out: 256 -> 267
                ro = 0
                for m in range(3):
                    msz = OUT_M[m]
                    p = ps.tile([msz, CH], dt.float32)
                    for i in range(2):
                        nc.tensor.matmul(
                            p[:], WR(f"outT_{i}_{m}"), h6[i][:],
                            start=(i == 0), stop=(i == 1),
                        )
                    o = apool.tile([msz, CH], dt.float32, name=f"o_{m}")
                    nc.scalar.activation(o[:], p[:], AF.Identity,
                                         bias=W32(f"outb_{m}"))
                    nc.sync.dma_start(out=reconT[ro:ro + msz, col], in_=o[:])
                    ro += msz

            nc.sync.dma_start(out=musq[:, :], in_=musq_buf[:])
            nc.sync.dma_start(out=maxv[:, :], in_=maxv_buf[:])
            nc.sync.dma_start(out=counts[:, :], in_=counts_buf[:])

    nc.compile()
    return nc, lay, R0, NW


def _pack_blob(inputs, lay, NW):
    wb = np.zeros((128, NW), np.float32)

    def put(nm, arr):
        p, o, w = lay[nm]
        a = np.asarray(arr, np.float32)
        assert a.shape == (p, w), (nm, a.shape, (p, w))
        wb[0:p, o:o + w] = a

    fc1T = np.asarray(inputs["fc1_w"], np.float32).T  # [534, 256]
    offs = np.cumsum([0] + S_SIZES)
    for i in range(6):
        for m in range(2):
            put(f"fc1T_{i}_{m}", fc1T[offs[i]:offs[i + 1], m * 128:(m + 1) * 128])
    fc2T = np.asarray(inputs["fc2_w"], np.float32).T
    fc3T = np.asarray(inputs["fc3_w"], np.float32).T
    for i in range(2):
        for m in range(2):
            put(f"fc2T_{i}_{m}", fc2T[i * 128:(i + 1) * 128, m * 128:(m + 1) * 128])
            put(f"fc3T_{i}_{m}", fc3T[i * 128:(i + 1) * 128, m * 128:(m + 1) * 128])
    muT = np.asarray(inputs["mu_w"], np.float32).T  # [256, 64]
    for i in range(2):
        put(f"muT_{i}", muT[i * 128:(i + 1) * 128])
    embed = np.asarray(inputs["embed"], np.float32)  # [64, 1024]
    ee = np.zeros((65, 1024), np.float32)
    ee[0:64] = embed
    ee[64] = (-0.5 * np.sum(embed.astype(np.float64) ** 2, axis=0)).astype(np.float32)
    put("embed_ext", ee)
    put("I128", np.eye(128, dtype=np.float32))
    put("iota", (np.arange(8)[None, :] * 128
                 + np.arange(128)[:, None]).astype(np.float32))
    for m in range(2):
        put(f"fc1b_{m}", np.asarray(inputs["fc1_b"], np.float32)[m * 128:(m + 1) * 128, None])
        put(f"fc2b_{m}", np.asarray(inputs["fc2_b"], np.float32)[m * 128:(m + 1) * 128, None])
        put(f"fc3b_{m}", np.asarray(inputs["fc3_b"], np.float32)[m * 128:(m + 1) * 128, None])
        put(f"fc4b_{m}", np.asarray(inputs["fc4_b"], np.float32)[m * 128:(m + 1) * 128, None])
        put(f"fc5b_{m}", np.asarray(inputs["fc5_b"], np.float32)[m * 128:(m + 1) * 128, None])
        put(f"fc6b_{m}", np.asarray(inputs["fc6_b"], np.float32)[m * 128:(m + 1) * 128, None])
    put("mub", np.asarray(inputs["mu_b"], np.float32)[:, None])
    ob = np.asarray(inputs["out_b"], np.float32)
    mo = 0
    for m in range(3):
        put(f"outb_{m}", ob[mo:mo + OUT_M[m], None])
        mo += OUT_M[m]
    embedT = embed.T  # [1024, 64]
    for i in range(8):
        put(f"embedT_{i}", embedT[i * 128:(i + 1) * 128])
    fc4T = np.asarray(inputs["fc4_w"], np.float32).T  # [331, 256]
    k4 = [fc4T[0:64], fc4T[64:192], fc4T[192:320], fc4T[320:331]]
    for ki in range(4):
        for m in range(2):
            put(f"fc4T_{ki}_{m}", k4[ki][:, m * 128:(m + 1) * 128])
    fc5T = np.asarray(inputs["fc5_w"], np.float32).T
    fc6T = np.asarray(inputs["fc6_w"], np.float32).T
    for i in range(2):
        for m in range(2):
            put(f"fc5T_{i}_{m}", fc5T[i * 128:(i + 1) * 128, m * 128:(m + 1) * 128])
            put(f"fc6T_{i}_{m}", fc6T[i * 128:(i + 1) * 128, m * 128:(m + 1) * 128])
    outT = np.asarray(inputs["out_w"], np.float32).T  # [256, 267]
    for i in range(2):
        mo = 0
        for m in range(3):
            put(f"outT_{i}_{m}", outT[i * 128:(i + 1) * 128, mo:mo + OUT_M[m]])
            mo += OUT_M[m]
    put("ones", np.ones((128, 128), np.float32))
    return wb


LAST_RESULT = None


def kernel(**inputs):
    global LAST_RESULT
    from concourse import bass_utils

    if "nc" not in _CACHE:
        _CACHE["nc"] = _build_nc()
    nc, lay, R0, NW = _CACHE["nc"]

    x = np.asarray(inputs["x"], np.float32)
    c = np.asarray(inputs["c"], np.float32)
    wb = _pack_blob(inputs, lay, NW)

    in_maps = []
    for i in range(NCORES):
        sl = slice(i * RPC, (i + 1) * RPC)
        in_maps.append({
            "xT": np.ascontiguousarray(x[sl].T),
            "cT": np.ascontiguousarray(c[sl].T),
            "wb": wb,
        })
    res = bass_utils.run_bass_kernel_spmd(nc, in_maps, core_ids=list(range(NCORES)))
    LAST_RESULT = res

    recon = np.concatenate(
        [np.ascontiguousarray(r["reconT"].T) for r in res.results], axis=0
    ).astype(np.float32)
    musq_s = sum(float(r["musq"].astype(np.float64).sum()) for r in res.results)
    maxv_s = sum(float(r["maxv"].astype(np.float64).sum()) for r in res.results)
    loss = (musq_s - 2.0 * maxv_s) / (B * L)
    cnt = np.zeros(KC, np.float64)
    for r in res.results:
        cb = r["counts"].astype(np.float64)             # [128, NCHUNK*8]
        cc = cb.reshape(128, NCHUNK, 8).sum(axis=1)     # [128, 8]
        cnt += cc.T.reshape(KC)                         # code = cc*128 + k
    p = cnt / B
    perp = np.exp(-np.sum(p * np.log(p + 1e-10)))
    return recon, np.float32(loss), np.float32(perp)


# revision 25
# speedup vs baseline: 1.1707x; 1.1707x over previous
import sys

sys.path.insert(0, "/opt/trn_rl_repo")
import numpy as np

try:
    import antenv.axon_hooks  # noqa: F401
except ImportError:
    # bass_utils hard-imports this when BASS_TRACE is set; stub it so the
    # run degrades to no-trace instead of crashing.
    import types

    _m = types.ModuleType("antenv.axon_hooks")
    _m._hook = None
    _m.get_axon_ntff_profile_hook = lambda: _m._hook

    def _set_hook(h):
        _m._hook = h

    _m.set_axon_ntff_profile_hook = _set_hook
    import antenv

    sys.modules["antenv.axon_hooks"] = _m
    antenv.axon_hooks = _m

B = 65536
FD = 267
H = 256
L = 64
KC = 1024
NCORES = 8
RPC = B // NCORES          # rows per core: 8192
CH = 512                   # rows per chunk
NCHUNK = RPC // CH         # 16
NSUB = CH // 128           # 4

# encoder input s=[x;c] (534 rows) partition-tile layout; all tiles
# partition-0 aligned (engines need 32-aligned partition starts)
S_SIZES = [128, 128, 11, 128, 128, 11]
# t0=x[0:128] t1=x[128:256] t2=x[256:267] t3=c[0:128] t4=c[128:256] t5=c[256:267]
# decoder K-chunks: q_st(64), c[0:128), c[128:256), c[256:267)
DEC_K = [64, 128, 128, 11]
OUT_M = [128, 128, 11]


def _blob_layout():
    lay = {}
    off = 0

    def add(nm, p, w):
        nonlocal off
        lay[nm] = (p, off, w)
        off += w

    # ---- fp32 region ----
    for i in range(2):
        add(f"muT_{i}", 128, 64)
    add("embed_ext", 65, 1024)
    add("I128", 128, 128)
    add("iota", 128, 8)
    for m in range(2):
        add(f"fc1b_{m}", 128, 1)
        add(f"fc2b_{m}", 128, 1)
        add(f"fc3b_{m}", 128, 1)
        add(f"fc5b_{m}", 128, 1)
        add(f"fc4b_{m}", 128, 1)
        add(f"fc6b_{m}", 128, 1)
    add("mub", 64, 1)
    for m in range(3):
        add(f"outb_{m}", OUT_M[m], 1)
    r0 = off
    # ---- f32r region (rounded on-chip via one Act copy) ----
    for i in range(6):
        for m in range(2):
            add(f"fc1T_{i}_{m}", S_SIZES[i], 128)
    for i in range(2):
        for m in range(2):
            add(f"fc2T_{i}_{m}", 128, 128)
    for i in range(2):
        for m in range(2):
            add(f"fc3T_{i}_{m}", 128, 128)
    for i in range(8):
        add(f"embedT_{i}", 128, 64)
    for ki in range(4):
        for m in range(2):
            add(f"fc4T_{ki}_{m}", DEC_K[ki], 128)
    for i in range(2):
        for m in range(2):
            add(f"fc5T_{i}_{m}", 128, 128)
    for i in range(2):
        for m in range(2):
            add(f"fc6T_{i}_{m}", 128, 128)
    for i in range(2):
        for m in range(3):
            add(f"outT_{i}_{m}", 128, OUT_M[m])
    add("ones", 128, 128)
    return lay, r0, off


_CACHE = {}


def _build_nc():
    import concourse.bacc as bacc
    import concourse.tile as tile
    import concourse.mybir as mybir

    dt = mybir.dt
    AF = mybir.ActivationFunctionType
    ALU = mybir.AluOpType
    AX = mybir.AxisListType

    lay, R0, NW = _blob_layout()
    nc = bacc.Bacc()
    xT = nc.declare_dram_parameter("xT", [FD, RPC], dt.float32, isOutput=False)
    cT = nc.declare_dram_parameter("cT", [FD, RPC], dt.float32, isOutput=False)
    wb = nc.declare_dram_parameter("wb", [128, NW], dt.float32, isOutput=False)
    reconT = nc.declare_dram_parameter("reconT", [FD, RPC], dt.float32, isOutput=True)
    musq = nc.declare_dram_parameter("musq", [L, NCHUNK], dt.float32, isOutput=True)
    maxv = nc.declare_dram_parameter(
        "maxv", [128, NCHUNK * NSUB], dt.float32, isOutput=True
    )
    counts = nc.declare_dram_parameter(
        "counts", [128, NCHUNK * 8], dt.float32, isOutput=True
    )

    with tile.TileContext(nc) as tc:
        with tc.tile_pool(name="const", bufs=1) as cpool, \
             tc.tile_pool(name="act", bufs=2) as apool, \
             tc.tile_pool(name="small", bufs=2) as spool, \
             tc.tile_pool(name="stat", bufs=1) as stpool, \
             tc.tile_pool(name="ps", bufs=2, space="PSUM") as ps, \
             tc.tile_pool(name="psq", bufs=1, space="PSUM") as psq, \
             tc.tile_pool(name="pss", bufs=2, space="PSUM") as pss:

            wb_sb = cpool.tile([128, NW], dt.float32)
            nc.sync.dma_start(out=wb_sb[:], in_=wb[:, :])
            WRW = NW - R0
            wr_sb = cpool.tile([128, WRW], dt.float32r)
            nc.scalar.copy(out=wr_sb[:], in_=wb_sb[:, R0:NW])

            def W32(nm):
                p, o, w = lay[nm]
                return wb_sb[0:p, o:o + w]

            def WR(nm):
                p, o, w = lay[nm]
                return wr_sb[0:p, o - R0:o - R0 + w]

            musq_buf = stpool.tile([L, NCHUNK], dt.float32)
            maxv_buf = stpool.tile([128, NCHUNK * NSUB], dt.float32)
            counts_buf = stpool.tile([128, NCHUNK * 8], dt.float32)

            for j in range(NCHUNK):
                col = slice(j * CH, (j + 1) * CH)
                t = [apool.tile([S_SIZES[i], CH], dt.float32, name=f"s_{i}")
                     for i in range(6)]
                nc.sync.dma_start(out=t[0][:], in_=xT[0:128, col])
                nc.sync.dma_start(out=t[1][:], in_=xT[128:256, col])
                nc.sync.dma_start(out=t[2][:], in_=xT[256:267, col])
                nc.sync.dma_start(out=t[3][:], in_=cT[0:128, col])
                nc.sync.dma_start(out=t[4][:], in_=cT[128:256, col])
                nc.sync.dma_start(out=t[5][:], in_=cT[256:267, col])
                # cast inputs to f32r (split across vector + scalar engines)
                tr = [apool.tile([S_SIZES[i], CH], dt.float32r, name=f"sr_{i}")
                      for i in range(6)]
                for i in range(3):
                    nc.vector.tensor_copy(out=tr[i][:], in_=t[i][:])
                for i in range(3, 6):
                    nc.scalar.copy(out=tr[i][:], in_=t[i][:])

                # encoder L1: [534]->256, relu (f32r)
                h1 = []
                for m in range(2):
                    p = ps.tile([128, CH], dt.float32)
                    for i in range(6):
                        nc.tensor.matmul(
                            p[:], WR(f"fc1T_{i}_{m}"), tr[i][:],
                            start=(i == 0), stop=(i == 5),
                        )
                    h = apool.tile([128, CH], dt.float32r, name=f"h1_{m}")
                    nc.scalar.activation(h[:], p[:], AF.Relu, bias=W32(f"fc1b_{m}"))
                    h1.append(h)
                # L2 (f32r)
                h2 = []
                for m in range(2):
                    p = ps.tile([128, CH], dt.float32)
                    for i in range(2):
                        nc.tensor.matmul(
                            p[:], WR(f"fc2T_{i}_{m}"), h1[i][:],
                            start=(i == 0), stop=(i == 1),
                        )
                    h = apool.tile([128, CH], dt.float32r, name=f"h2_{m}")
                    nc.scalar.activation(h[:], p[:], AF.Relu, bias=W32(f"fc2b_{m}"))
                    h2.append(h)
                # L3 (f32r matmul, fp32 output h3 -> feeds fp32 mu matmul)
                h3 = []
                for m in range(2):
                    p = ps.tile([128, CH], dt.float32)
                    for i in range(2):
                        nc.tensor.matmul(
                            p[:], WR(f"fc3T_{i}_{m}"), h2[i][:],
                            start=(i == 0), stop=(i == 1),
                        )
                    h = apool.tile([128, CH], dt.float32, name=f"h3_{m}")
                    nc.scalar.activation(h[:], p[:], AF.Relu, bias=W32(f"fc3b_{m}"))
                    h3.append(h)
                # mu: 256->64 (no relu)
                pmb = psq.tile([128, CH], dt.float32, name="pmb")
                pmu = pmb[0:64, :]
                for i in range(2):
                    nc.tensor.matmul(
                        pmu, W32(f"muT_{i}"), h3[i][:],
                        start=(i == 0), stop=(i == 1),
                    )
                muext = apool.tile([65, CH], dt.float32)
                nc.scalar.activation(muext[0:64, :], pmu, AF.Identity,
                                     bias=W32("mub"))
                nc.vector.memset(muext[64:65, :], 1.0)
                sqscr = apool.tile([64, CH], dt.float32)
                nc.scalar.activation(
                    sqscr[:], pmu, AF.Square, bias=W32("mub"),
                    accum_out=musq_buf[:, j:j + 1],
                )

                # scores + argmax per 128-row subtile
                diag = apool.tile([128, CH], dt.float32r)
                emb = W32("embed_ext")
                for s in range(NSUB):
                    psc = pss.tile([128, 1024], dt.float32)
                    lhs = muext[:, s * 128:(s + 1) * 128]
                    nc.tensor.matmul(psc[:, 0:512], lhs, emb[:, 0:512],
                                     start=True, stop=True)
                    nc.tensor.matmul(psc[:, 512:1024], lhs, emb[:, 512:1024],
                                     start=True, stop=True)
                    mx8 = spool.tile([128, 8], dt.float32)
                    nc.vector.max(out=mx8[:], in_=psc[:])
                    ix8 = spool.tile([128, 8], dt.uint32)
                    nc.vector.max_index(out=ix8[:], in_max=mx8[:], in_values=psc[:])
                    cix = j * NSUB + s
                    nc.vector.tensor_copy(out=maxv_buf[:, cix:cix + 1],
                                          in_=mx8[:, 0:1])
                    ixf = spool.tile([128, 1], dt.float32)
                    nc.vector.tensor_copy(out=ixf[:], in_=ix8[:, 0:1])
                    nc.vector.tensor_scalar(
                        out=diag[:, s * 128:(s + 1) * 128], in0=W32("I128"),
                        scalar1=ixf[:], scalar2=None, op0=ALU.mult,
                    )
                # broadcast idx to [128, CH] (reuses pmu's slot; pmu is dead)
                pbc = psq.tile([128, CH], dt.float32, name="pmb")
                nc.tensor.matmul(pbc[:], WR("ones"), diag[:], start=True, stop=True)

                # onehot chunks, counts, quantize
                pq = psq.tile([64, CH], dt.float32, name="pq")
                iota = W32("iota")
                for cc in range(8):
                    oh = apool.tile([128, CH], dt.float32r)
                    nc.vector.tensor_scalar(
                        out=oh[:], in0=pbc[:], scalar1=iota[:, cc:cc + 1],
                        scalar2=None, op0=ALU.is_equal,
                    )
                    ci = j * 8 + cc
                    nc.vector.tensor_reduce(
                        out=counts_buf[:, ci:ci + 1], in_=oh[:], axis=AX.X,
                        op=ALU.add,
                    )
                    nc.tensor.matmul(
                        pq[:], WR(f"embedT_{cc}"), oh[:],
                        start=(cc == 0), stop=(cc == 7),
                    )
                qst = apool.tile([64, CH], dt.float32r)
                nc.scalar.copy(out=qst[:], in_=pq[:])
                dec_rhs = [qst, tr[3], tr[4], tr[5]]

                # decoder L1
                h4 = []
                for m in range(2):
                    p = ps.tile([128, CH], dt.float32)
                    for ki in range(4):
                        nc.tensor.matmul(
                            p[:], WR(f"fc4T_{ki}_{m}"), dec_rhs[ki][:],
                            start=(ki == 0), stop=(ki == 3),
                        )
                    h = apool.tile([128, CH], dt.float32r, name=f"h4_{m}")
                    nc.scalar.activation(h[:], p[:], AF.Relu, bias=W32(f"fc4b_{m}"))
                    h4.append(h)
                # L2
                h5 = []
                for m in range(2):
                    p = ps.tile([128, CH], dt.float32)
                    for i in range(2):
                        nc.tensor.matmul(
                            p[:], WR(f"fc5T_{i}_{m}"), h4[i][:],
                            start=(i == 0), stop=(i == 1),
                        )
                    h = apool.tile([128, CH], dt.float32r, name=f"h5_{m}")
                    nc.scalar.activation(h[:], p[:], AF.Relu, bias=W32(f"fc5b_{m}"))
                    h5.append(h)
                # L3: fc6
                h6 = []
                for m in range(2):
                    p = ps.tile([128, CH], dt.float32)
                    for i in range(2):
                        nc.tensor.matmul(
                            p[:], WR(f"fc6T_{i}_{m}"), h5[i][:],
                            start=(i == 0), stop=(i == 1),
                        )
                    h = apool.tile([128, CH], dt.float32r, name=f"h6_{m}")
                    nc.scalar.activation(h[:], p[:], AF.Relu, bias=W32(f"fc6b_{m}"))
                    h6.append(h)
                # L4 out: 256 -> 267
                ro = 0
                for m in range(3):
                    msz = OUT_M[m]
                    p = ps.tile([msz, CH], dt.float32)
                    for i in range(2):
                        nc.tensor.matmul(
                            p[:], WR(f"outT_{i}_{m}"), h6[i][:],
                            start=(i == 0), stop=(i == 1),
                        )
                    o = apool.tile([msz, CH], dt.float32, name=f"o_{m}")
                    nc.scalar.activation(o[:], p[:], AF.Identity,
                                         bias=W32(f"outb_{m}"))
                    nc.sync.dma_start(out=reconT[ro:ro + msz, col], in_=o[:])
                    ro += msz

            nc.sync.dma_start(out=musq[:, :], in_=musq_buf[:])
            nc.sync.dma_start(out=maxv[:, :], in_=maxv_buf[:])
            nc.sync.dma_start(out=counts[:, :], in_=counts_buf[:])

    nc.compile()
    return nc, lay, R0, NW


def _pack_blob(inputs, lay, NW):
    wb = np.zeros((128, NW), np.float32)

    def put(nm, arr):
        p, o, w = lay[nm]
        a = np.asarray(arr, np.float32)
        assert a.shape == (p, w), (nm, a.shape, (p, w))
        wb[0:p, o:o + w] = a

    fc1T = np.asarray(inputs["fc1_w"], np.float32).T  # [534, 256]
    offs = np.cumsum([0] + S_SIZES)
    for i in range(6):
        for m in range(2):
            put(f"fc1T_{i}_{m}", fc1T[offs[i]:offs[i + 1], m * 128:(m + 1) * 128])
    fc2T = np.asarray(inputs["fc2_w"], np.float32).T
    fc3T = np.asarray(inputs["fc3_w"], np.float32).T
    for i in range(2):
        for m in range(2):
            put(f"fc2T_{i}_{m}", fc2T[i * 128:(i + 1) * 128, m * 128:(m + 1) * 128])
            put(f"fc3T_{i}_{m}", fc3T[i * 128:(i + 1) * 128, m * 128:(m + 1) * 128])
    muT = np.asarray(inputs["mu_w"], np.float32).T  # [256, 64]
    for i in range(2):
        put(f"muT_{i}", muT[i * 128:(i + 1) * 128])
    embed = np.asarray(inputs["embed"], np.float32)  # [64, 1024]
    ee = np.zeros((65, 1024), np.float32)
    ee[0:64] = embed
    ee[64] = (-0.5 * np.sum(embed.astype(np.float64) ** 2, axis=0)).astype(np.float32)
    put("embed_ext", ee)
    put("I128", np.eye(128, dtype=np.float32))
    put("iota", (np.arange(8)[None, :] * 128
                 + np.arange(128)[:, None]).astype(np.float32))
    for m in range(2):
        put(f"fc1b_{m}", np.asarray(inputs["fc1_b"], np.float32)[m * 128:(m + 1) * 128, None])
        put(f"fc2b_{m}", np.asarray(inputs["fc2_b"], np.float32)[m * 128:(m + 1) * 128, None])
        put(f"fc3b_{m}", np.asarray(inputs["fc3_b"], np.float32)[m * 128:(m + 1) * 128, None])
        put(f"fc4b_{m}", np.asarray(inputs["fc4_b"], np.float32)[m * 128:(m + 1) * 128, None])
        put(f"fc5b_{m}", np.asarray(inputs["fc5_b"], np.float32)[m * 128:(m + 1) * 128, None])
        put(f"fc6b_{m}", np.asarray(inputs["fc6_b"], np.float32)[m * 128:(m + 1) * 128, None])
    put("mub", np.asarray(inputs["mu_b"], np.float32)[:, None])
    ob = np.asarray(inputs["out_b"], np.float32)
    mo = 0
    for m in range(3):
        put(f"outb_{m}", ob[mo:mo + OUT_M[m], None])
        mo += OUT_M[m]
    embedT = embed.T  # [1024, 64]
    for i in range(8):
        put(f"embedT_{i}", embedT[i * 128:(i + 1) * 128])
    fc4T = np.asarray(inputs["fc4_w"], np.float32).T  # [331, 256]
    k4 = [fc4T[0:64], fc4T[64:192], fc4T[192:320], fc4T[320:331]]
    for ki in range(4):
        for m in range(2):
            put(f"fc4T_{ki}_{m}", k4[ki][:, m * 128:(m + 1) * 128])
    fc5T = np.asarray(inputs["fc5_w"], np.float32).T
    fc6T = np.asarray(inputs["fc6_w"], np.float32).T
    for i in range(2):
        for m in range(2):
            put(f"fc5T_{i}_{m}", fc5T[i * 128:(i + 1) * 128, m * 128:(m + 1) * 128])
            put(f"fc6T_{i}_{m}", fc6T[i * 128:(i + 1) * 128, m * 128:(m + 1) * 128])
    outT = np.asarray(inputs["out_w"], np.float32).T  # [256, 267]
    for i in range(2):
        mo = 0
        for m in range(3):
            put(f"outT_{i}_{m}", outT[i * 128:(i + 1) * 128, mo:mo + OUT_M[m]])
            mo += OUT_M[m]
    put("ones", np.ones((128, 128), np.float32))
    return wb


LAST_RESULT = None


def kernel(**inputs):
    global LAST_RESULT
    from concourse import bass_utils

    if "nc" not in _CACHE:
        _CACHE["nc"] = _build_nc()
    nc, lay, R0, NW = _CACHE["nc"]

    x = np.asarray(inputs["x"], np.float32)
    c = np.asarray(inputs["c"], np.float32)
    wb = _pack_blob(inputs, lay, NW)

    in_maps = []
    for i in range(NCORES):
        sl = slice(i * RPC, (i + 1) * RPC)
        in_maps.append({
            "xT": np.ascontiguousarray(x[sl].T),
            "cT": np.ascontiguousarray(c[sl].T),
            "wb": wb,
        })
    res = bass_utils.run_bass_kernel_spmd(nc, in_maps, core_ids=list(range(NCORES)))
    LAST_RESULT = res

    recon = np.concatenate(
        [np.ascontiguousarray(r["reconT"].T) for r in res.results], axis=0
    ).astype(np.float32)
    musq_s = sum(float(r["musq"].astype(np.float64).sum()) for r in res.results)
    maxv_s = sum(float(r["maxv"].astype(np.float64).sum()) for r in res.results)
    loss = (musq_s - 2.0 * maxv_s) / (B * L)
    cnt = np.zeros(KC, np.float64)
    for r in res.results:
        cb = r["counts"].astype(np.float64)             # [128, NCHUNK*8]
        cc = cb.reshape(128, NCHUNK, 8).sum(axis=1)     # [128, 8]
        cnt += cc.T.reshape(KC)                         # code = cc*128 + k
    p = cnt / B
    perp = np.exp(-np.sum(p * np.log(p + 1e-10)))
    return recon, np.float32(loss), np.float32(perp)


# revision 37
# speedup vs baseline: 1.9811x; 1.6922x over previous
import sys

sys.path.insert(0, "/opt/trn_rl_repo")
import numpy as np

try:
    import antenv.axon_hooks  # noqa: F401
except ImportError:
    # bass_utils hard-imports this when BASS_TRACE is set; stub it so the
    # run degrades to no-trace instead of crashing.
    import types

    _m = types.ModuleType("antenv.axon_hooks")
    _m._hook = None
    _m.get_axon_ntff_profile_hook = lambda: _m._hook

    def _set_hook(h):
        _m._hook = h

    _m.set_axon_ntff_profile_hook = _set_hook
    import antenv

    sys.modules["antenv.axon_hooks"] = _m
    antenv.axon_hooks = _m

B = 65536
FD = 267
H = 256
L = 64
KC = 1024
NCORES = 8
RPC = B // NCORES          # rows per core: 8192
CH = 512                   # rows per chunk
NCHUNK = RPC // CH         # 16
NSUB = CH // 128           # 4

# encoder input s=[x;c] (534 rows) partition-tile layout; all tiles
# partition-0 aligned (engines need 32-aligned partition starts)
S_SIZES = [128, 128, 11, 128, 128, 11]
# t0=x[0:128] t1=x[128:256] t2=x[256:267] t3=c[0:128] t4=c[128:256] t5=c[256:267]
# decoder K-chunks: q_st(64), c[0:128), c[128:256), c[256:267)
DEC_K = [64, 128, 128, 11]
OUT_M = [128, 128, 11]


def _blob_layout():
    lay = {}
    off = 0

    def add(nm, p, w):
        nonlocal off
        lay[nm] = (p, off, w)
        off += w

    # ---- fp32 region ----
    for i in range(2):
        add(f"muT_{i}", 128, 64)
    add("embed_ext", 65, 1024)
    add("I128", 128, 128)
    for m in range(2):
        add(f"fc1b_{m}", 128, 1)
        add(f"fc2b_{m}", 128, 1)
        add(f"fc3b_{m}", 128, 1)
        add(f"fc5b_{m}", 128, 1)
        add(f"fc4b_{m}", 128, 1)
        add(f"fc6b_{m}", 128, 1)
    add("mub", 64, 1)
    for m in range(3):
        add(f"outb_{m}", OUT_M[m], 1)
    r0 = off
    # ---- f32r region (rounded on-chip via one Act copy) ----
    for i in range(6):
        for m in range(2):
            add(f"fc1T_{i}_{m}", S_SIZES[i], 128)
    for i in range(2):
        for m in range(2):
            add(f"fc2T_{i}_{m}", 128, 128)
    for i in range(2):
        for m in range(2):
            add(f"fc3T_{i}_{m}", 128, 128)
    for ki in range(4):
        for m in range(2):
            add(f"fc4T_{ki}_{m}", DEC_K[ki], 128)
    for i in range(2):
        for m in range(2):
            add(f"fc5T_{i}_{m}", 128, 128)
    for i in range(2):
        for m in range(2):
            add(f"fc6T_{i}_{m}", 128, 128)
    for i in range(2):
        for m in range(3):
            add(f"outT_{i}_{m}", 128, OUT_M[m])
    return lay, r0, off


_CACHE = {}


def _build_nc():
    import concourse.bacc as bacc
    import concourse.bass as bass
    import concourse.tile as tile
    import concourse.mybir as mybir

    dt = mybir.dt
    AF = mybir.ActivationFunctionType
    ALU = mybir.AluOpType
    AX = mybir.AxisListType

    lay, R0, NW = _blob_layout()
    nc = bacc.Bacc()
    xT = nc.declare_dram_parameter("xT", [FD, RPC], dt.float32, isOutput=False)
    cT = nc.declare_dram_parameter("cT", [FD, RPC], dt.float32, isOutput=False)
    wb = nc.declare_dram_parameter("wb", [128, NW], dt.float32, isOutput=False)
    reconT = nc.declare_dram_parameter("reconT", [FD, RPC], dt.float32, isOutput=True)
    musq = nc.declare_dram_parameter("musq", [L, NCHUNK], dt.float32, isOutput=True)
    maxv = nc.declare_dram_parameter(
        "maxv", [128, NCHUNK * NSUB], dt.float32, isOutput=True
    )
    idxs = nc.declare_dram_parameter(
        "idxs", [128, NCHUNK * NSUB], dt.uint32, isOutput=True
    )
    embT_d = nc.declare_dram_parameter("embT", [KC, L], dt.float32, isOutput=False)

    with tile.TileContext(nc) as tc:
        with tc.tile_pool(name="const", bufs=1) as cpool, \
             tc.tile_pool(name="act", bufs=2) as apool, \
             tc.tile_pool(name="small", bufs=2) as spool, \
             tc.tile_pool(name="stat", bufs=1) as stpool, \
             tc.tile_pool(name="ps", bufs=2, space="PSUM") as ps, \
             tc.tile_pool(name="psq", bufs=1, space="PSUM") as psq, \
             tc.tile_pool(name="pss", bufs=2, space="PSUM") as pss:

            wb_sb = cpool.tile([128, NW], dt.float32)
            nc.sync.dma_start(out=wb_sb[:], in_=wb[:, :])
            WRW = NW - R0
            wr_sb = cpool.tile([128, WRW], dt.float32r)
            nc.scalar.copy(out=wr_sb[:], in_=wb_sb[:, R0:NW])

            def W32(nm):
                p, o, w = lay[nm]
                return wb_sb[0:p, o:o + w]

            def WR(nm):
                p, o, w = lay[nm]
                return wr_sb[0:p, o - R0:o - R0 + w]

            musq_buf = stpool.tile([L, NCHUNK], dt.float32)
            maxv_buf = stpool.tile([128, NCHUNK * NSUB], dt.float32)
            idx_buf = stpool.tile([128, NCHUNK * NSUB], dt.uint32)

            def enc_chunk(j):
                col = slice(j * CH, (j + 1) * CH)
                t = [apool.tile([S_SIZES[i], CH], dt.float32, name=f"s_{i}")
                     for i in range(6)]
                nc.sync.dma_start(out=t[0][:], in_=xT[0:128, col])
                nc.sync.dma_start(out=t[1][:], in_=xT[128:256, col])
                nc.sync.dma_start(out=t[2][:], in_=xT[256:267, col])
                nc.sync.dma_start(out=t[3][:], in_=cT[0:128, col])
                nc.sync.dma_start(out=t[4][:], in_=cT[128:256, col])
                nc.sync.dma_start(out=t[5][:], in_=cT[256:267, col])
                # cast inputs to f32r (split across vector + scalar engines)
                tr = [apool.tile([S_SIZES[i], CH], dt.float32r, name=f"sr_{i}")
                      for i in range(6)]
                for i in range(3):
                    nc.vector.tensor_copy(out=tr[i][:], in_=t[i][:])
                for i in range(3, 6):
                    nc.scalar.copy(out=tr[i][:], in_=t[i][:])

                # encoder L1: [534]->256, relu (f32r)
                h1 = []
                for m in range(2):
                    p = ps.tile([128, CH], dt.float32, name="p")
                    for i in range(6):
                        nc.tensor.matmul(
                            p[:], WR(f"fc1T_{i}_{m}"), tr[i][:],
                            start=(i == 0), stop=(i == 5),
                        )
                    h = apool.tile([128, CH], dt.float32r, name=f"h1_{m}")
                    nc.scalar.activation(h[:], p[:], AF.Relu, bias=W32(f"fc1b_{m}"))
                    h1.append(h)
                # L2 (f32r)
                h2 = []
                for m in range(2):
                    p = ps.tile([128, CH], dt.float32, name="p")
                    for i in range(2):
                        nc.tensor.matmul(
                            p[:], WR(f"fc2T_{i}_{m}"), h1[i][:],
                            start=(i == 0), stop=(i == 1),
                        )
                    h = apool.tile([128, CH], dt.float32r, name=f"h2_{m}")
                    nc.scalar.activation(h[:], p[:], AF.Relu, bias=W32(f"fc2b_{m}"))
                    h2.append(h)
                # L3 (f32r matmul, fp32 output h3 -> feeds fp32 mu matmul)
                h3 = []
                for m in range(2):
                    p = ps.tile([128, CH], dt.float32, name="p")
                    for i in range(2):
                        nc.tensor.matmul(
                            p[:], WR(f"fc3T_{i}_{m}"), h2[i][:],
                            start=(i == 0), stop=(i == 1),
                        )
                    h = apool.tile([128, CH], dt.float32, name=f"h3_{m}")
                    nc.scalar.activation(h[:], p[:], AF.Relu, bias=W32(f"fc3b_{m}"))
                    h3.append(h)
                # mu: 256->64 (no relu)
                pmb = psq.tile([128, CH], dt.float32, name="pmb")
                pmu = pmb[0:64, :]
                for i in range(2):
                    nc.tensor.matmul(
                        pmu, W32(f"muT_{i}"), h3[i][:],
                        start=(i == 0), stop=(i == 1),
                    )
                muext = apool.tile([65, CH], dt.float32, name="muext")
                nc.scalar.activation(muext[0:64, :], pmu, AF.Identity,
                                     bias=W32("mub"))
                nc.vector.memset(muext[64:65, :], 1.0)
                sqscr = apool.tile([64, CH], dt.float32, name="sqscr")
                nc.scalar.activation(
                    sqscr[:], pmu, AF.Square, bias=W32("mub"),
                    accum_out=musq_buf[:, j:j + 1],
                )

                # scores + argmax + quantize-gather per 128-row subtile
                gats = []
                emb = W32("embed_ext")
                for s in range(NSUB):
                    psc = pss.tile([128, 1024], dt.float32, name="psc")
                    lhs = muext[:, s * 128:(s + 1) * 128]
                    nc.tensor.matmul(psc[:, 0:512], lhs, emb[:, 0:512],
                                     start=True, stop=True)
                    nc.tensor.matmul(psc[:, 512:1024], lhs, emb[:, 512:1024],
                                     start=True, stop=True)
                    mx8 = spool.tile([128, 8], dt.float32, name="mx8")
                    nc.vector.max(out=mx8[:], in_=psc[:])
                    ix8 = spool.tile([128, 8], dt.uint32, name=f"ix8_{s}")
                    nc.vector.max_index(out=ix8[:], in_max=mx8[:], in_values=psc[:])
                    cix = j * NSUB + s
                    nc.vector.tensor_copy(out=maxv_buf[:, cix:cix + 1],
                                          in_=mx8[:, 0:1])
                    nc.vector.tensor_copy(out=idx_buf[:, cix:cix + 1],
                                          in_=ix8[:, 0:1])
                    gat = apool.tile([128, L], dt.float32, name=f"gat_{s}")
                    nc.gpsimd.indirect_dma_start(
                        out=gat[:], out_offset=None, in_=embT_d[:, :],
                        in_offset=bass.IndirectOffsetOnAxis(ap=ix8[:, 0:1],
                                                            axis=0),
                    )
                    gats.append(gat)
                return col, tr, gats

            def dec_chunk(state):
                col, tr, gats = state
                # quantize rows gathered as [128, 64]; PE-transpose to [64, CH]
                ptq = psq.tile([64, CH], dt.float32, name="ptq")
                for s in range(NSUB):
                    nc.tensor.transpose(out=ptq[:, s * 128:(s + 1) * 128],
                                        in_=gats[s][:], identity=W32("I128"))
                qst = apool.tile([64, CH], dt.float32r, name="qst")
                nc.scalar.copy(out=qst[:], in_=ptq[:])
                dec_rhs = [qst, tr[3], tr[4], tr[5]]

                # decoder L1
                h4 = []
                for m in range(2):
                    p = ps.tile([128, CH], dt.float32, name="p")
                    for ki in range(4):
                        nc.tensor.matmul(
                            p[:], WR(f"fc4T_{ki}_{m}"), dec_rhs[ki][:],
                            start=(ki == 0), stop=(ki == 3),
                        )
                    h = apool.tile([128, CH], dt.float32r, name=f"h4_{m}")
                    nc.scalar.activation(h[:], p[:], AF.Relu, bias=W32(f"fc4b_{m}"))
                    h4.append(h)
                # L2
                h5 = []
                for m in range(2):
                    p = ps.tile([128, CH], dt.float32, name="p")
                    for i in range(2):
                        nc.tensor.matmul(
                            p[:], WR(f"fc5T_{i}_{m}"), h4[i][:],
                            start=(i == 0), stop=(i == 1),
                        )
                    h = apool.tile([128, CH], dt.float32r, name=f"h5_{m}")
                    nc.scalar.activation(h[:], p[:], AF.Relu, bias=W32(f"fc5b_{m}"))
                    h5.append(h)
                # L3: fc6
                h6 = []
                for m in range(2):
                    p = ps.tile([128, CH], dt.float32, name="p")
                    for i in range(2):
                        nc.tensor.matmul(
                            p[:], WR(f"fc6T_{i}_{m}"), h5[i][:],
                            start=(i == 0), stop=(i == 1),
                        )
                    h = apool.tile([128, CH], dt.float32r, name=f"h6_{m}")
                    nc.scalar.activation(h[:], p[:], AF.Relu, bias=W32(f"fc6b_{m}"))
                    h6.append(h)
                # L4 out: 256 -> 267
                ro = 0
                for m in range(3):
                    msz = OUT_M[m]
                    p = ps.tile([msz, CH], dt.float32, name="p")
                    for i in range(2):
                        nc.tensor.matmul(
                            p[:], WR(f"outT_{i}_{m}"), h6[i][:],
                            start=(i == 0), stop=(i == 1),
                        )
                    o = apool.tile([msz, CH], dt.float32, name=f"o_{m}")
                    nc.scalar.activation(o[:], p[:], AF.Identity,
                                         bias=W32(f"outb_{m}"))
                    nc.sync.dma_start(out=reconT[ro:ro + msz, col], in_=o[:])
                    ro += msz

            # software pipeline: decoder for chunk j-1 is issued after
            # encoder+scores of chunk j so the quantize gather (indirect DMA)
            # completes before the PE reaches the transpose.
            prev = None
            for j in range(NCHUNK + 1):
                nxt = enc_chunk(j) if j < NCHUNK else None
                if prev is not None:
                    dec_chunk(prev)
                prev = nxt

            nc.sync.dma_start(out=musq[:, :], in_=musq_buf[:])
            nc.sync.dma_start(out=maxv[:, :], in_=maxv_buf[:])
            nc.sync.dma_start(out=idxs[:, :], in_=idx_buf[:])

    nc.compile()
    return nc, lay, R0, NW


def _pack_blob(inputs, lay, NW):
    wb = np.zeros((128, NW), np.float32)

    def put(nm, arr):
        p, o, w = lay[nm]
        a = np.asarray(arr, np.float32)
        assert a.shape == (p, w), (nm, a.shape, (p, w))
        wb[0:p, o:o + w] = a

    fc1T = np.asarray(inputs["fc1_w"], np.float32).T  # [534, 256]
    offs = np.cumsum([0] + S_SIZES)
    for i in range(6):
        for m in range(2):
            put(f"fc1T_{i}_{m}", fc1T[offs[i]:offs[i + 1], m * 128:(m + 1) * 128])
    fc2T = np.asarray(inputs["fc2_w"], np.float32).T
    fc3T = np.asarray(inputs["fc3_w"], np.float32).T
    for i in range(2):
        for m in range(2):
            put(f"fc2T_{i}_{m}", fc2T[i * 128:(i + 1) * 128, m * 128:(m + 1) * 128])
            put(f"fc3T_{i}_{m}", fc3T[i * 128:(i + 1) * 128, m * 128:(m + 1) * 128])
    muT = np.asarray(inputs["mu_w"], np.float32).T  # [256, 64]
    for i in range(2):
        put(f"muT_{i}", muT[i * 128:(i + 1) * 128])
    embed = np.asarray(inputs["embed"], np.float32)  # [64, 1024]
    ee = np.zeros((65, 1024), np.float32)
    ee[0:64] = embed
    ee[64] = (-0.5 * np.sum(embed.astype(np.float64) ** 2, axis=0)).astype(np.float32)
    put("embed_ext", ee)
    put("I128", np.eye(128, dtype=np.float32))
    for m in range(2):
        put(f"fc1b_{m}", np.asarray(inputs["fc1_b"], np.float32)[m * 128:(m + 1) * 128, None])
        put(f"fc2b_{m}", np.asarray(inputs["fc2_b"], np.float32)[m * 128:(m + 1) * 128, None])
        put(f"fc3b_{m}", np.asarray(inputs["fc3_b"], np.float32)[m * 128:(m + 1) * 128, None])
        put(f"fc4b_{m}", np.asarray(inputs["fc4_b"], np.float32)[m * 128:(m + 1) * 128, None])
        put(f"fc5b_{m}", np.asarray(inputs["fc5_b"], np.float32)[m * 128:(m + 1) * 128, None])
        put(f"fc6b_{m}", np.asarray(inputs["fc6_b"], np.float32)[m * 128:(m + 1) * 128, None])
    put("mub", np.asarray(inputs["mu_b"], np.float32)[:, None])
    ob = np.asarray(inputs["out_b"], np.float32)
    mo = 0
    for m in range(3):
        put(f"outb_{m}", ob[mo:mo + OUT_M[m], None])
        mo += OUT_M[m]
    fc4T = np.asarray(inputs["fc4_w"], np.float32).T  # [331, 256]
    k4 = [fc4T[0:64], fc4T[64:192], fc4T[192:320], fc4T[320:331]]
    for ki in range(4):
        for m in range(2):
            put(f"fc4T_{ki}_{m}", k4[ki][:, m * 128:(m + 1) * 128])
    fc5T = np.asarray(inputs["fc5_w"], np.float32).T
    fc6T = np.asarray(inputs["fc6_w"], np.float32).T
    for i in range(2):
        for m in range(2):
            put(f"fc5T_{i}_{m}", fc5T[i * 128:(i + 1) * 128, m * 128:(m + 1) * 128])
            put(f"fc6T_{i}_{m}", fc6T[i * 128:(i + 1) * 128, m * 128:(m + 1) * 128])
    outT = np.asarray(inputs["out_w"], np.float32).T  # [256, 267]
    for i in range(2):
        mo = 0
        for m in range(3):
            put(f"outT_{i}_{m}", outT[i * 128:(i + 1) * 128, mo:mo + OUT_M[m]])
            mo += OUT_M[m]
    return wb


LAST_RESULT = None


def kernel(**inputs):
    global LAST_RESULT
    from concourse import bass_utils

    if "nc" not in _CACHE:
        _CACHE["nc"] = _build_nc()
    nc, lay, R0, NW = _CACHE["nc"]

    x = np.asarray(inputs["x"], np.float32)
    c = np.asarray(inputs["c"], np.float32)
    wb = _pack_blob(inputs, lay, NW)
    embT = np.ascontiguousarray(np.asarray(inputs["embed"], np.float32).T)

    in_maps = []
    for i in range(NCORES):
        sl = slice(i * RPC, (i + 1) * RPC)
        in_maps.append({
            "xT": np.ascontiguousarray(x[sl].T),
            "cT": np.ascontiguousarray(c[sl].T),
            "wb": wb,
            "embT": embT,
        })
    res = bass_utils.run_bass_kernel_spmd(nc, in_maps, core_ids=list(range(NCORES)))
    LAST_RESULT = res

    recon = np.concatenate(
        [np.ascontiguousarray(r["reconT"].T) for r in res.results], axis=0
    ).astype(np.float32)
    musq_s = sum(float(r["musq"].astype(np.float64).sum()) for r in res.results)
    maxv_s = sum(float(r["maxv"].astype(np.float64).sum()) for r in res.results)
    loss = (musq_s - 2.0 * maxv_s) / (B * L)
    cnt = np.zeros(KC, np.int64)
    for r in res.results:
        cnt += np.bincount(r["idxs"].reshape(-1).astype(np.int64), minlength=KC)
    p = cnt.astype(np.float64) / B
    perp = np.exp(-np.sum(p * np.log(p + 1e-10)))
    return recon, np.float32(loss), np.float32(perp)
